# revision 16
# baseline (speedup 1.0000x reference)
"""Distributed causal-self-attention kernel for one TRN2 chip (8 NeuronCores).

Reference math (T = D = N = 4096, faithful to the oracle):
    q = x @ Wq + bq ; k = x @ Wk + bk ; v = x @ Wv + bv      # [T, D]
    scores = (q @ k.T) / sqrt(D)                             # [T, T]
    p = softmax(scores, axis=-1)
    out = p @ v.T            # i.e. out[i, j] = sum_k p[i, k] * v[j, k]

Distribution: sequence-parallel over T. Core c owns rows R_c = [512c, 512(c+1)).
Each core computes qT/kT/vT for its own rows in TRANSPOSED layout [D, 512],
all-gathers kT and vT (so every core holds full K/V), then computes its
512-row slice of the output. Compute is bf16 on the TensorEngine with fp32
PSUM accumulation (measured end-to-end rel err ~5e-3 vs the fp32 oracle).

The transposed-projection layout puts every matmul contraction on the
partition axis with zero on-chip transposes:
    scoresT tile [j,i] = kT_chunk.T @ qT_chunk   (keys j on partitions)
    E = exp(scoresT / 64)        (scores are ~N(0,1); no max-subtraction needed)
    sums[i] = sum_j E[j, i]      (matmul with a ones vector)
    out tile [i, jout] = sum_k E[k, i] * vT[k, jout], scaled by 1/sums[i]

Matmul emission: the first matmul of each accumulation group covers the full
512-col PSUM bank (start=True initializes has_written); the remaining 31
contraction chunks stream as 2x256-col halves.  At full clock a 1:1
LDWEIGHTS:MATMUL pattern at N=512 exposes ~50ns of weight-load per matmul;
the 2x256 split fully hides the loads (measured 109ns/mm vs 132 floor at the
power-throttled 1.95GHz PE clock this kernel runs at chip-wide).

Startup/transition scheduling: the first xT chunk + first W chunk are issued
as dedicated DMAs before the bulk loads, the bulk is spread across the three
DMA-issuing queues (sync/scalar/gpsimd), and the first K-block (phase 2) and
first V-block (phase 3) are prefetched during the preceding phase so the
TensorEngine never waits at phase boundaries.
"""

import os
import sys

import numpy as np

for _p in ("/opt/trn_rl_repo", "/root/.axon_site/_ro/trn_rl_repo"):
    if os.path.isdir(_p) and _p not in sys.path:
        sys.path.insert(0, _p)

import ml_dtypes

P = 128                 # partitions
T = 4096                # seq len == d == input feature dim
NCORES = 8
S = T // NCORES         # 512 rows owned per core
KO = T // P             # 32 contraction chunks of 128
NB = T // S             # 8 key/value blocks of 512
NSUB = S // P           # 4 row-subtiles per core
SCALE = 1.0 / 64.0      # 1/sqrt(4096)

_BF16 = ml_dtypes.bfloat16


def _accum_matmuls(nc, ps, lhsT_of_ko, rhs_of_ko):
    """Emit the 32-chunk accumulation into one 512-col PSUM bank.

    ko=0 is a full-width N=512 matmul with start=True (clears the bank's
    has_written bits); ko>=1 stream as 2x256 halves so the per-chunk
    LDWEIGHTS fully hides under the in-flight matmuls.
    """
    nc.tensor.matmul(ps[:], lhsT_of_ko(0), rhs_of_ko(0, 0, S),
                     start=True, stop=False)
    for ko in range(1, KO):
        last = ko == KO - 1
        for h in (0, 1):
            nc.tensor.matmul(
                ps[:, h * 256:(h + 1) * 256],
                lhsT_of_ko(ko),
                rhs_of_ko(ko, h * 256, (h + 1) * 256),
                start=False,
                stop=last,
            )


def _build_program():
    import concourse.mybir as mybir
    from concourse import bacc
    from concourse.tile import TileContext

    f32 = mybir.dt.float32
    bf16 = mybir.dt.bfloat16
    Ident = mybir.ActivationFunctionType.Identity
    Exp = mybir.ActivationFunctionType.Exp

    nc = bacc.Bacc(
        "TRN2",
        target_bir_lowering=False,
        debug=False,
        enable_asserts=False,
        num_devices=NCORES,
    )

    # Per-core inputs. xT is x[R_c, :].T. Weights are pre-tiled on the host:
    # W_t[dt, p, ko*128 + f] = W[ko*128 + p, dt*128 + f], so the lhsT chunk
    # for output d-tile `dt`, contraction chunk `ko` is the contiguous slice
    # W_t[dt][:, ko*128:(ko+1)*128]. b3 packs the biases as
    # b3[p, t*32 + dt] = b_t[dt*128 + p] for t in (q, k, v).
    xT = nc.dram_tensor("xT", [T, S], bf16, kind="ExternalInput")
    Wq = nc.dram_tensor("Wq", [KO, P, T], bf16, kind="ExternalInput")
    Wk = nc.dram_tensor("Wk", [KO, P, T], bf16, kind="ExternalInput")
    Wv = nc.dram_tensor("Wv", [KO, P, T], bf16, kind="ExternalInput")
    b3 = nc.dram_tensor("b3", [P, 3 * KO], f32, kind="ExternalInput")
    out = nc.dram_tensor("out", [S, T], f32, kind="ExternalOutput")

    rg = [list(range(NCORES))]

    with TileContext(nc) as tc:
        with tc.tile_pool(name="dram", bufs=1, space="DRAM") as dram:
            kT_bounce = dram.tile([T, S], bf16)
            vT_bounce = dram.tile([T, S], bf16)
            # AllGather concatenates rank shards on axis 0:
            # kTg[c*T + d, r] = k[c*512 + r, d]
            kTg = dram.tile([NCORES * T, S], bf16, addr_space="Shared")
            vTg = dram.tile([NCORES * T, S], bf16, addr_space="Shared")

            with tc.tile_pool(name="persist", bufs=1) as persist, \
                 tc.tile_pool(name="Ep", bufs=1) as Ep:
                # E_sb[p, jo, i] = exp(scores[i_global, jo*128 + p] / 64)
                E_sb = Ep.tile([P, KO, S], bf16)
                qTp = tc.alloc_tile_pool(name="qTp", bufs=1)
                qT_sb = qTp.tile([P, KO, S], bf16)        # qT[d, i], released after QK
                ones_sb = persist.tile([P, 1], f32)
                b3_sb = persist.tile([P, 3 * KO], f32)
                recip_sb = persist.tile([P, NSUB], f32)   # 1/softmax-denominator
                acc_sb = persist.tile([P, S], f32)        # per-partition partial sums of E
                nc.vector.memset(ones_sb[:], 1.0)

                # Dedicated 1-buf pool for the first K block so its load can
                # prefetch while phase 1 still runs (pools reserve their SBUF
                # upfront, so the main block pool only exists in phases 2+3).
                kb0p = tc.alloc_tile_pool(name="kb0p", bufs=1)
                kb_first = [None]
                vb_first = [None]

                def _block_dma(tile, gathered, blk, n_dmas):
                    src = gathered[blk * T:(blk + 1) * T, :].rearrange(
                        "(ko p) f -> p ko f", p=P)
                    step = KO // n_dmas
                    for i in range(n_dmas):
                        nc.sync.dma_start(
                            tile[:, i * step:(i + 1) * step, :],
                            src[:, i * step:(i + 1) * step, :],
                        )

                # ---------- Phase 1: projections kT, vT, qT ----------
                with tc.tile_pool(name="xTp", bufs=1) as xTp, \
                     tc.tile_pool(name="wp", bufs=6) as wp, \
                     tc.tile_pool(name="kvstage", bufs=6) as kvstage, \
                     tc.tile_pool(name="ppsum", bufs=6, space="PSUM") as ppsum:
                    xT_sb = xTp.tile([P, KO, S], bf16)
                    xr = xT[:].rearrange("(ko p) f -> p ko f", p=P)
                    # Critical path: chunk 0 alone (first matmul group input),
                    # then the bulk split across the three DMA queues so the
                    # first dt-groups are never starved.
                    nc.sync.dma_start(xT_sb[:, 0, :], xr[:, 0, :])
                    nc.scalar.dma_start(xT_sb[:, 1:4, :], xr[:, 1:4, :])
                    nc.sync.dma_start(xT_sb[:, 4:12, :], xr[:, 4:12, :])
                    nc.scalar.dma_start(xT_sb[:, 12:20, :], xr[:, 12:20, :])
                    nc.sync.dma_start(xT_sb[:, 20:28, :], xr[:, 20:28, :])
                    nc.scalar.dma_start(xT_sb[:, 28:32, :], xr[:, 28:32, :])
                    nc.scalar.dma_start(b3_sb[:], b3[:])

                    # k first, then v (so their all-gathers overlap the rest
                    # of the projection compute), then q (stays in SBUF).
                    for wi, (W, bounce, boff) in enumerate((
                        (Wk, kT_bounce, KO),
                        (Wv, vT_bounce, 2 * KO),
                        (Wq, None, 0),
                    )):
                        for dt in range(KO):
                            w_sb = wp.tile([P, T], bf16, tag="w")
                            # The first three W tiles ride the gpsimd queue so
                            # they don't sit behind the xT bulk on sync/scalar.
                            # Everything later must stay OFF gpsimd: the
                            # collective_compute occupies that queue for the
                            # full gather duration and would head-of-line
                            # block anything emitted after it.
                            if wi == 0 and dt == 0:
                                nc.gpsimd.dma_start(w_sb[:, 0:128], W[dt][:, 0:128])
                                nc.gpsimd.dma_start(w_sb[:, 128:512], W[dt][:, 128:512])
                                nc.gpsimd.dma_start(w_sb[:, 512:4096], W[dt][:, 512:4096])
                            elif wi == 0 and dt in (1, 2):
                                nc.gpsimd.dma_start(w_sb[:], W[dt])
                            else:
                                nc.sync.dma_start(w_sb[:], W[dt])
                            ps = ppsum.tile([P, S], f32, tag="pp")
                            _accum_matmuls(
                                nc, ps,
                                lambda ko, w_sb=w_sb: w_sb[:, ko * P:(ko + 1) * P],
                                lambda ko, lo, hi: xT_sb[:, ko, lo:hi],
                            )
                            bias = b3_sb[:, boff + dt:boff + dt + 1]
                            if wi == 2 and dt == 24:
                                # Prefetch the first K block late in phase 1:
                                # the k gather is done by now, so these DMAs
                                # issue immediately (emitting them right after
                                # the gather would park a semaphore wait at
                                # the head of the sync queue and block every
                                # phase-1 DMA behind it).
                                kb_first[0] = kb0p.tile([P, KO, S], bf16, name="kb0")
                                _block_dma(kb_first[0], kTg, 0, 4)
                            if bounce is None:
                                nc.scalar.activation(qT_sb[:, dt, :], ps[:], Ident, bias=bias)
                            else:
                                st = kvstage.tile([P, S], bf16, tag="st")
                                nc.scalar.activation(st[:], ps[:], Ident, bias=bias)
                                nc.sync.dma_start(bounce[dt * P:(dt + 1) * P, :], st[:])
                        if wi == 0:
                            nc.gpsimd.collective_compute(
                                "AllGather", mybir.AluOpType.bypass,
                                replica_groups=rg, ins=[kT_bounce[:]], outs=[kTg[:]],
                            )
                        elif wi == 1:
                            nc.gpsimd.collective_compute(
                                "AllGather", mybir.AluOpType.bypass,
                                replica_groups=rg, ins=[vT_bounce[:]], outs=[vTg[:]],
                            )

                # ---------- Phase 2: scoresT -> E = exp(scoresT/64) ----------
                bpool = tc.alloc_tile_pool(name="blocks", bufs=3)
                with tc.tile_pool(name="qkpsum", bufs=6, space="PSUM") as qkpsum, \
                     tc.tile_pool(name="spsum", bufs=2, space="PSUM") as spsum:
                    for jb in range(NB):
                        if jb == 0:
                            kb = kb_first[0]
                        else:
                            kb = bpool.tile([P, KO, S], bf16, tag="blk")
                            _block_dma(kb, kTg, jb, 4)
                        for js in range(NSUB):
                            ps = qkpsum.tile([P, S], f32, tag="qk")
                            _accum_matmuls(
                                nc, ps,
                                lambda ko, kb=kb, js=js: kb[:, ko, js * P:(js + 1) * P],
                                lambda ko, lo, hi: qT_sb[:, ko, lo:hi],
                            )
                            nc.scalar.activation(
                                E_sb[:, jb * NSUB + js, :], ps[:], Exp, scale=SCALE)
                        if jb == NB - 1:
                            # Prefetch the first V block during the last QK
                            # block (the v gather is long done by then, so no
                            # head-of-line wait on the sync queue).
                            vb_first[0] = bpool.tile([P, KO, S], bf16, tag="blk", name="vb0")
                            _block_dma(vb_first[0], vTg, 0, 4)
                        Evb = E_sb[:, jb * NSUB:(jb + 1) * NSUB, :].rearrange(
                            "p ko i -> p i ko")
                        if jb == 0:
                            nc.vector.reduce_sum(
                                acc_sb[:], Evb, axis=mybir.AxisListType.X)
                        else:
                            pt = persist.tile([P, S], f32, tag="pt", bufs=2)
                            nc.vector.reduce_sum(
                                pt[:], Evb, axis=mybir.AxisListType.X)
                            nc.vector.tensor_add(acc_sb[:], acc_sb[:], pt[:])

                    for ii in range(NSUB):
                        sp = spsum.tile([P, 1], f32, tag="sum")
                        nc.tensor.matmul(
                            sp[:], acc_sb[:, ii * P:(ii + 1) * P], ones_sb[:],
                            start=True, stop=True)
                        nc.vector.reciprocal(recip_sb[:, ii:ii + 1], sp[:])

                # ---------- Phase 3: out = (E.T @ vT) / sums ----------
                with tc.tile_pool(name="pvpsum", bufs=6, space="PSUM") as pvpsum, \
                     tc.tile_pool(name="ostage", bufs=4) as ostage:
                    for vb in range(NB):
                        if vb == 0:
                            vbt = vb_first[0]
                        else:
                            vbt = bpool.tile([P, KO, S], bf16, tag="blk")
                            _block_dma(vbt, vTg, vb, 4)
                        for ii in range(NSUB):
                            ps = pvpsum.tile([P, S], f32, tag="pv")
                            _accum_matmuls(
                                nc, ps,
                                lambda ko, ii=ii: E_sb[:, ko, ii * P:(ii + 1) * P],
                                lambda ko, lo, hi, vbt=vbt: vbt[:, ko, lo:hi],
                            )
                            ot = ostage.tile([P, S], f32, tag="ot")
                            nc.vector.tensor_scalar_mul(
                                ot[:], ps[:], recip_sb[:, ii:ii + 1])
                            if vb == NB - 1:
                                h = S // 2
                                nc.sync.dma_start(
                                    out[ii * P:(ii + 1) * P, vb * S:vb * S + h],
                                    ot[:, :h])
                                nc.sync.dma_start(
                                    out[ii * P:(ii + 1) * P, vb * S + h:(vb + 1) * S],
                                    ot[:, h:])
                            else:
                                nc.sync.dma_start(
                                    out[ii * P:(ii + 1) * P, vb * S:(vb + 1) * S], ot[:])
                bpool.release()
                kb0p.release()
                qTp.release()
    nc.compile()
    return nc


def _tile_weight(W):
    # W_t[dt, p, ko*128 + f] = W[ko*128 + p, dt*128 + f]
    W4 = np.asarray(W, dtype=np.float32).reshape(KO, P, KO, P)
    return np.ascontiguousarray(W4.transpose(2, 1, 0, 3).reshape(KO, P, T)).astype(_BF16)


def _prepare_in_maps(inputs):
    x = np.asarray(inputs["x"], dtype=np.float32)
    Wqt = _tile_weight(inputs["Wq"])
    Wkt = _tile_weight(inputs["Wk"])
    Wvt = _tile_weight(inputs["Wv"])
    b3 = np.ascontiguousarray(
        np.concatenate(
            [np.asarray(inputs[k], np.float32).reshape(KO, P).T for k in ("bq", "bk", "bv")],
            axis=1,
        )
    )
    in_maps = []
    for c in range(NCORES):
        xT_c = np.ascontiguousarray(x[c * S:(c + 1) * S, :].T).astype(_BF16)
        in_maps.append({"xT": xT_c, "Wq": Wqt, "Wk": Wkt, "Wv": Wvt, "b3": b3})
    return in_maps


def _run(inputs, trace=False, **spmd_kwargs):
    from concourse.bass_utils import run_bass_kernel_spmd

    nc = _build_program()
    in_maps = _prepare_in_maps(inputs)
    res = run_bass_kernel_spmd(
        nc, in_maps, list(range(NCORES)), trace=trace, **spmd_kwargs)
    out = np.concatenate(
        [np.asarray(res.results[c]["out"], dtype=np.float32) for c in range(NCORES)],
        axis=0,
    )
    return out, res


def kernel(**inputs):
    out, _ = _run(inputs, trace=False)
    return out


# revision 22
# speedup vs baseline: 1.0749x; 1.0749x over previous
"""Distributed causal-self-attention kernel for one TRN2 chip (8 NeuronCores).

Reference math (T = D = N = 4096, faithful to the oracle):
    q = x @ Wq + bq ; k = x @ Wk + bk ; v = x @ Wv + bv      # [T, D]
    scores = (q @ k.T) / sqrt(D)                             # [T, T]
    p = softmax(scores, axis=-1)
    out = p @ v.T            # i.e. out[i, j] = sum_k p[i, k] * v[j, k]

Distribution: sequence-parallel over T. Core c owns rows R_c = [512c, 512(c+1)).
Each core computes qT/kT/vT for its own rows in TRANSPOSED layout [D, 512],
all-gathers kT and vT (so every core holds full K/V), then computes its
512-row slice of the output. Compute is bf16 on the TensorEngine with fp32
PSUM accumulation (measured end-to-end rel err ~5e-3 vs the fp32 oracle).

The transposed-projection layout puts every matmul contraction on the
partition axis with zero on-chip transposes:
    scoresT tile [j,i] = kT_chunk.T @ qT_chunk   (keys j on partitions)
    E = exp(scoresT / 64)        (scores are ~N(0,1); no max-subtraction needed)
    sums[i] = sum_j E[j, i]      (matmul with a ones vector)
    out tile [i, jout] = sum_k E[k, i] * vT[k, jout], scaled by 1/sums[i]

Matmul emission: the first matmul of each accumulation group covers the full
512-col PSUM bank (start=True initializes has_written); the remaining 31
contraction chunks stream as 2x256-col halves.  At full clock a 1:1
LDWEIGHTS:MATMUL pattern at N=512 exposes ~50ns of weight-load per matmul;
the 2x256 split fully hides the loads (measured 109ns/mm vs 132 floor at the
power-throttled 1.95GHz PE clock this kernel runs at chip-wide).

Startup/transition scheduling: the first xT chunk + first W chunk are issued
as dedicated DMAs before the bulk loads, the bulk is spread across the three
DMA-issuing queues (sync/scalar/gpsimd), and the first K-block (phase 2) and
first V-block (phase 3) are prefetched during the preceding phase so the
TensorEngine never waits at phase boundaries.
"""

import os
import sys

import numpy as np

for _p in ("/opt/trn_rl_repo", "/root/.axon_site/_ro/trn_rl_repo"):
    if os.path.isdir(_p) and _p not in sys.path:
        sys.path.insert(0, _p)

import ml_dtypes

P = 128                 # partitions
T = 4096                # seq len == d == input feature dim
NCORES = 8
S = T // NCORES         # 512 rows owned per core
KO = T // P             # 32 contraction chunks of 128
NB = T // S             # 8 key/value blocks of 512
NSUB = S // P           # 4 row-subtiles per core
SCALE = 1.0 / 64.0      # 1/sqrt(4096)

_BF16 = ml_dtypes.bfloat16


def _accum_matmuls(nc, ps, lhsT_of_ko, rhs_of_ko):
    """Emit the 32-chunk accumulation into one 512-col PSUM bank.

    ko=0 is a full-width N=512 matmul with start=True (clears the bank's
    has_written bits); ko>=1 stream as 2x256 halves so the per-chunk
    LDWEIGHTS fully hides under the in-flight matmuls.
    """
    nc.tensor.matmul(ps[:], lhsT_of_ko(0), rhs_of_ko(0, 0, S),
                     start=True, stop=False)
    for ko in range(1, KO):
        last = ko == KO - 1
        for h in (0, 1):
            nc.tensor.matmul(
                ps[:, h * 256:(h + 1) * 256],
                lhsT_of_ko(ko),
                rhs_of_ko(ko, h * 256, (h + 1) * 256),
                start=False,
                stop=last,
            )


def _build_program():
    import concourse.mybir as mybir
    from concourse import bacc
    from concourse.tile import TileContext

    f32 = mybir.dt.float32
    bf16 = mybir.dt.bfloat16
    Ident = mybir.ActivationFunctionType.Identity
    Exp = mybir.ActivationFunctionType.Exp

    nc = bacc.Bacc(
        "TRN2",
        target_bir_lowering=False,
        debug=False,
        enable_asserts=False,
        num_devices=NCORES,
    )

    # Per-core inputs. xT is x[R_c, :].T. Weights are pre-tiled on the host:
    # W_t[dt, p, ko*128 + f] = W[ko*128 + p, dt*128 + f], so the lhsT chunk
    # for output d-tile `dt`, contraction chunk `ko` is the contiguous slice
    # W_t[dt][:, ko*128:(ko+1)*128]. b3 packs the biases as
    # b3[p, t*32 + dt] = b_t[dt*128 + p] for t in (q, k, v).
    xT = nc.dram_tensor("xT", [T, S], bf16, kind="ExternalInput")
    Wq = nc.dram_tensor("Wq", [KO, P, T], bf16, kind="ExternalInput")
    Wk = nc.dram_tensor("Wk", [KO, P, T], bf16, kind="ExternalInput")
    Wv = nc.dram_tensor("Wv", [KO, P, T], bf16, kind="ExternalInput")
    b3 = nc.dram_tensor("b3", [P, 3 * KO], f32, kind="ExternalInput")
    out = nc.dram_tensor("out", [S, T], f32, kind="ExternalOutput")

    rg = [list(range(NCORES))]

    with TileContext(nc) as tc:
        with tc.tile_pool(name="dram", bufs=1, space="DRAM") as dram:
            kT_bounce = dram.tile([T, S], bf16)
            vT_bounce = dram.tile([T, S], bf16)
            # AllGather concatenates rank shards on axis 0:
            # kTg[c*T + d, r] = k[c*512 + r, d]
            kTg = dram.tile([NCORES * T, S], bf16, addr_space="Shared")
            vTg = dram.tile([NCORES * T, S], bf16, addr_space="Shared")

            with tc.tile_pool(name="persist", bufs=1) as persist, \
                 tc.tile_pool(name="Ep", bufs=1) as Ep:
                # E_sb[p, jo, i] = exp(scores[i_global, jo*128 + p] / 64)
                E_sb = Ep.tile([P, KO, S], bf16)
                qTp = tc.alloc_tile_pool(name="qTp", bufs=1)
                qT_sb = qTp.tile([P, KO, S], bf16)        # qT[d, i], released after QK
                ones_sb = persist.tile([P, 1], f32)
                b3_sb = persist.tile([P, 3 * KO], f32)
                recip_sb = persist.tile([P, NSUB], f32)   # 1/softmax-denominator
                acc_sb = persist.tile([P, S], f32)        # per-partition partial sums of E
                nc.vector.memset(ones_sb[:], 1.0)

                # Dedicated 1-buf pool for the first K block so its load can
                # prefetch while phase 1 still runs (pools reserve their SBUF
                # upfront, so the main block pool only exists in phases 2+3).
                kb0p = tc.alloc_tile_pool(name="kb0p", bufs=1)
                kb_first = [None]
                vb_first = [None]

                def _block_dma(tile, gathered, blk, n_dmas, eng=None):
                    src = gathered[blk * T:(blk + 1) * T, :].rearrange(
                        "(ko p) f -> p ko f", p=P)
                    step = KO // n_dmas
                    for i in range(n_dmas):
                        (eng or nc.sync).dma_start(
                            tile[:, i * step:(i + 1) * step, :],
                            src[:, i * step:(i + 1) * step, :],
                        )

                # ---------- Phase 1: projections kT, vT, qT ----------
                with tc.tile_pool(name="xTp", bufs=1) as xTp, \
                     tc.tile_pool(name="wp", bufs=6) as wp, \
                     tc.tile_pool(name="kvstage", bufs=6) as kvstage, \
                     tc.tile_pool(name="ppsum", bufs=6, space="PSUM") as ppsum:
                    xT_sb = xTp.tile([P, KO, S], bf16)
                    xr = xT[:].rearrange("(ko p) f -> p ko f", p=P)
                    # Critical path: chunk 0 alone (first matmul group input),
                    # then the bulk split across the three DMA queues so the
                    # first dt-groups are never starved.
                    for c4 in range(4):
                        nc.gpsimd.dma_start(
                            xT_sb[:, 0, c4 * P:(c4 + 1) * P],
                            xr[:, 0, c4 * P:(c4 + 1) * P])
                    nc.sync.dma_start(xT_sb[:, 1, :], xr[:, 1, :])
                    for lo in range(2, 16, 2):
                        nc.sync.dma_start(
                            xT_sb[:, lo:lo + 2, :], xr[:, lo:lo + 2, :])
                    for lo in range(16, KO, 2):
                        nc.scalar.dma_start(
                            xT_sb[:, lo:lo + 2, :], xr[:, lo:lo + 2, :])
                    nc.scalar.dma_start(b3_sb[:], b3[:])

                    # k first, then v (so their all-gathers overlap the rest
                    # of the projection compute), then q (stays in SBUF).
                    for wi, (W, bounce, boff) in enumerate((
                        (Wk, kT_bounce, KO),
                        (Wv, vT_bounce, 2 * KO),
                        (Wq, None, 0),
                    )):
                        for dt in range(KO):
                            w_sb = wp.tile([P, T], bf16, tag="w")
                            if wi == 0 and dt == 0:
                                nc.gpsimd.dma_start(w_sb[:, 0:128], W[dt][:, 0:128])
                                nc.gpsimd.dma_start(w_sb[:, 128:256], W[dt][:, 128:256])
                                for lo, hi in ((256, 512), (512, 1024),
                                               (1024, 2048), (2048, 4096)):
                                    nc.sync.dma_start(w_sb[:, lo:hi], W[dt][:, lo:hi])
                            else:
                                nc.sync.dma_start(w_sb[:], W[dt])
                            ps = ppsum.tile([P, S], f32, tag="pp")
                            _accum_matmuls(
                                nc, ps,
                                lambda ko, w_sb=w_sb: w_sb[:, ko * P:(ko + 1) * P],
                                lambda ko, lo, hi: xT_sb[:, ko, lo:hi],
                            )
                            bias = b3_sb[:, boff + dt:boff + dt + 1]
                            if bounce is None:
                                nc.scalar.activation(qT_sb[:, dt, :], ps[:], Ident, bias=bias)
                            else:
                                st = kvstage.tile([P, S], bf16, tag="st")
                                nc.scalar.activation(st[:], ps[:], Ident, bias=bias)
                                nc.sync.dma_start(bounce[dt * P:(dt + 1) * P, :], st[:])
                        if wi == 0:
                            nc.gpsimd.collective_compute(
                                "AllGather", mybir.AluOpType.bypass,
                                replica_groups=rg, ins=[kT_bounce[:]], outs=[kTg[:]],
                            )
                            # Prefetch the first K block on the GPSIMD queue:
                            # it sits right behind the k collective there, so
                            # its wait-on-gather parks harmlessly (on sync it
                            # would head-of-line block the phase-1 W loads —
                            # the Tile scheduler hoists it early regardless of
                            # emission position).
                            kb_first[0] = kb0p.tile([P, KO, S], bf16, name="kb0")
                            _block_dma(kb_first[0], kTg, 0, 4, eng=nc.gpsimd)
                        elif wi == 1:
                            nc.gpsimd.collective_compute(
                                "AllGather", mybir.AluOpType.bypass,
                                replica_groups=rg, ins=[vT_bounce[:]], outs=[vTg[:]],
                            )

                # ---------- Phase 2: scoresT -> E = exp(scoresT/64) ----------
                bpool = tc.alloc_tile_pool(name="blocks", bufs=3)
                with tc.tile_pool(name="qkpsum", bufs=6, space="PSUM") as qkpsum, \
                     tc.tile_pool(name="spsum", bufs=2, space="PSUM") as spsum:
                    for jb in range(NB):
                        if jb == 0:
                            kb = kb_first[0]
                        else:
                            kb = bpool.tile([P, KO, S], bf16, tag="blk")
                            _block_dma(kb, kTg, jb, 4)
                        for js in range(NSUB):
                            ps = qkpsum.tile([P, S], f32, tag="qk")
                            _accum_matmuls(
                                nc, ps,
                                lambda ko, kb=kb, js=js: kb[:, ko, js * P:(js + 1) * P],
                                lambda ko, lo, hi: qT_sb[:, ko, lo:hi],
                            )
                            nc.scalar.activation(
                                E_sb[:, jb * NSUB + js, :], ps[:], Exp, scale=SCALE)
                        if jb == 0:
                            # Prefetch the first V block on the GPSIMD queue,
                            # where it sits right behind the v collective (see
                            # the K-block prefetch note above).
                            vb_first[0] = bpool.tile([P, KO, S], bf16, tag="blk", name="vb0")
                            _block_dma(vb_first[0], vTg, 0, 4, eng=nc.gpsimd)
                        Evb = E_sb[:, jb * NSUB:(jb + 1) * NSUB, :].rearrange(
                            "p ko i -> p i ko")
                        if jb == 0:
                            nc.vector.reduce_sum(
                                acc_sb[:], Evb, axis=mybir.AxisListType.X)
                        else:
                            pt = persist.tile([P, S], f32, tag="pt", bufs=2)
                            nc.vector.reduce_sum(
                                pt[:], Evb, axis=mybir.AxisListType.X)
                            nc.vector.tensor_add(acc_sb[:], acc_sb[:], pt[:])

                    for ii in range(NSUB):
                        sp = spsum.tile([P, 1], f32, tag="sum")
                        nc.tensor.matmul(
                            sp[:], acc_sb[:, ii * P:(ii + 1) * P], ones_sb[:],
                            start=True, stop=True)
                        nc.vector.reciprocal(recip_sb[:, ii:ii + 1], sp[:])

                # ---------- Phase 3: out = (E.T @ vT) / sums ----------
                with tc.tile_pool(name="pvpsum", bufs=6, space="PSUM") as pvpsum, \
                     tc.tile_pool(name="ostage", bufs=4) as ostage:
                    for vb in range(NB):
                        if vb == 0:
                            vbt = vb_first[0]
                        else:
                            vbt = bpool.tile([P, KO, S], bf16, tag="blk")
                            _block_dma(vbt, vTg, vb, 4)
                        for ii in range(NSUB):
                            ps = pvpsum.tile([P, S], f32, tag="pv")
                            _accum_matmuls(
                                nc, ps,
                                lambda ko, ii=ii: E_sb[:, ko, ii * P:(ii + 1) * P],
                                lambda ko, lo, hi, vbt=vbt: vbt[:, ko, lo:hi],
                            )
                            ot = ostage.tile([P, S], f32, tag="ot")
                            nc.vector.tensor_scalar_mul(
                                ot[:], ps[:], recip_sb[:, ii:ii + 1])
                            if vb == NB - 1:
                                h = S // 2
                                nc.sync.dma_start(
                                    out[ii * P:(ii + 1) * P, vb * S:vb * S + h],
                                    ot[:, :h])
                                nc.sync.dma_start(
                                    out[ii * P:(ii + 1) * P, vb * S + h:(vb + 1) * S],
                                    ot[:, h:])
                            else:
                                nc.sync.dma_start(
                                    out[ii * P:(ii + 1) * P, vb * S:(vb + 1) * S], ot[:])
                bpool.release()
                kb0p.release()
                qTp.release()
    nc.compile()
    return nc


def _tile_weight(W):
    # W_t[dt, p, ko*128 + f] = W[ko*128 + p, dt*128 + f]
    W4 = np.asarray(W, dtype=np.float32).reshape(KO, P, KO, P)
    return np.ascontiguousarray(W4.transpose(2, 1, 0, 3).reshape(KO, P, T)).astype(_BF16)


def _prepare_in_maps(inputs):
    x = np.asarray(inputs["x"], dtype=np.float32)
    Wqt = _tile_weight(inputs["Wq"])
    Wkt = _tile_weight(inputs["Wk"])
    Wvt = _tile_weight(inputs["Wv"])
    b3 = np.ascontiguousarray(
        np.concatenate(
            [np.asarray(inputs[k], np.float32).reshape(KO, P).T for k in ("bq", "bk", "bv")],
            axis=1,
        )
    )
    in_maps = []
    for c in range(NCORES):
        xT_c = np.ascontiguousarray(x[c * S:(c + 1) * S, :].T).astype(_BF16)
        in_maps.append({"xT": xT_c, "Wq": Wqt, "Wk": Wkt, "Wv": Wvt, "b3": b3})
    return in_maps


def _run(inputs, trace=False, **spmd_kwargs):
    from concourse.bass_utils import run_bass_kernel_spmd

    nc = _build_program()
    in_maps = _prepare_in_maps(inputs)
    res = run_bass_kernel_spmd(
        nc, in_maps, list(range(NCORES)), trace=trace, **spmd_kwargs)
    out = np.concatenate(
        [np.asarray(res.results[c]["out"], dtype=np.float32) for c in range(NCORES)],
        axis=0,
    )
    return out, res


def kernel(**inputs):
    out, _ = _run(inputs, trace=False)
    return out


# revision 23
# speedup vs baseline: 1.0795x; 1.0043x over previous
"""Distributed causal-self-attention kernel for one TRN2 chip (8 NeuronCores).

Reference math (T = D = N = 4096, faithful to the oracle):
    q = x @ Wq + bq ; k = x @ Wk + bk ; v = x @ Wv + bv      # [T, D]
    scores = (q @ k.T) / sqrt(D)                             # [T, T]
    p = softmax(scores, axis=-1)
    out = p @ v.T            # i.e. out[i, j] = sum_k p[i, k] * v[j, k]

Distribution: sequence-parallel over T. Core c owns rows R_c = [512c, 512(c+1)).
Each core computes qT/kT/vT for its own rows in TRANSPOSED layout [D, 512],
all-gathers kT and vT (so every core holds full K/V), then computes its
512-row slice of the output. Compute is bf16 on the TensorEngine with fp32
PSUM accumulation (measured end-to-end rel err ~5e-3 vs the fp32 oracle).

The transposed-projection layout puts every matmul contraction on the
partition axis with zero on-chip transposes:
    scoresT tile [j,i] = kT_chunk.T @ qT_chunk   (keys j on partitions)
    E = exp(scoresT / 64)        (scores are ~N(0,1); no max-subtraction needed)
    sums[i] = sum_j E[j, i]      (matmul with a ones vector)
    out tile [i, jout] = sum_k E[k, i] * vT[k, jout], scaled by 1/sums[i]

Matmul emission: the first matmul of each accumulation group covers the full
512-col PSUM bank (start=True initializes has_written); the remaining 31
contraction chunks stream as 2x256-col halves.  At full clock a 1:1
LDWEIGHTS:MATMUL pattern at N=512 exposes ~50ns of weight-load per matmul;
the 2x256 split fully hides the loads (measured 109ns/mm vs 132 floor at the
power-throttled 1.95GHz PE clock this kernel runs at chip-wide).

Startup/transition scheduling: the first xT chunk + first W chunk are issued
as dedicated DMAs before the bulk loads, the bulk is spread across the three
DMA-issuing queues (sync/scalar/gpsimd), and the first K-block (phase 2) and
first V-block (phase 3) are prefetched during the preceding phase so the
TensorEngine never waits at phase boundaries.
"""

import os
import sys

import numpy as np

for _p in ("/opt/trn_rl_repo", "/root/.axon_site/_ro/trn_rl_repo"):
    if os.path.isdir(_p) and _p not in sys.path:
        sys.path.insert(0, _p)

import ml_dtypes

P = 128                 # partitions
T = 4096                # seq len == d == input feature dim
NCORES = 8
S = T // NCORES         # 512 rows owned per core
KO = T // P             # 32 contraction chunks of 128
NB = T // S             # 8 key/value blocks of 512
NSUB = S // P           # 4 row-subtiles per core
SCALE = 1.0 / 64.0      # 1/sqrt(4096)

_BF16 = ml_dtypes.bfloat16


def _accum_matmuls(nc, ps, lhsT_of_ko, rhs_of_ko):
    """Emit the 32-chunk accumulation into one 512-col PSUM bank.

    ko=0 is a full-width N=512 matmul with start=True (clears the bank's
    has_written bits); ko>=1 stream as 2x256 halves so the per-chunk
    LDWEIGHTS fully hides under the in-flight matmuls.
    """
    nc.tensor.matmul(ps[:], lhsT_of_ko(0), rhs_of_ko(0, 0, S),
                     start=True, stop=False)
    for ko in range(1, KO):
        last = ko == KO - 1
        for h in (0, 1):
            nc.tensor.matmul(
                ps[:, h * 256:(h + 1) * 256],
                lhsT_of_ko(ko),
                rhs_of_ko(ko, h * 256, (h + 1) * 256),
                start=False,
                stop=last,
            )


def _build_program():
    import concourse.mybir as mybir
    from concourse import bacc
    from concourse.tile import TileContext

    f32 = mybir.dt.float32
    bf16 = mybir.dt.bfloat16
    Ident = mybir.ActivationFunctionType.Identity
    Exp = mybir.ActivationFunctionType.Exp

    nc = bacc.Bacc(
        "TRN2",
        target_bir_lowering=False,
        debug=False,
        enable_asserts=False,
        num_devices=NCORES,
    )

    # Per-core inputs. xT is x[R_c, :].T. Weights are pre-tiled on the host:
    # W_t[dt, p, ko*128 + f] = W[ko*128 + p, dt*128 + f], so the lhsT chunk
    # for output d-tile `dt`, contraction chunk `ko` is the contiguous slice
    # W_t[dt][:, ko*128:(ko+1)*128]. b3 packs the biases as
    # b3[p, t*32 + dt] = b_t[dt*128 + p] for t in (q, k, v).
    xT = nc.dram_tensor("xT", [T, S], bf16, kind="ExternalInput")
    Wq = nc.dram_tensor("Wq", [KO, P, T], bf16, kind="ExternalInput")
    Wk = nc.dram_tensor("Wk", [KO, P, T], bf16, kind="ExternalInput")
    Wv = nc.dram_tensor("Wv", [KO, P, T], bf16, kind="ExternalInput")
    b3 = nc.dram_tensor("b3", [P, 3 * KO], f32, kind="ExternalInput")
    out = nc.dram_tensor("out", [S, T], f32, kind="ExternalOutput")

    rg = [list(range(NCORES))]

    with TileContext(nc) as tc:
        with tc.tile_pool(name="dram", bufs=1, space="DRAM") as dram:
            kT_bounce = dram.tile([T, S], bf16)
            vT_bounce = dram.tile([T, S], bf16)
            # AllGather concatenates rank shards on axis 0:
            # kTg[c*T + d, r] = k[c*512 + r, d]
            kTg = dram.tile([NCORES * T, S], bf16, addr_space="Shared")
            vTg = dram.tile([NCORES * T, S], bf16, addr_space="Shared")

            with tc.tile_pool(name="persist", bufs=1) as persist, \
                 tc.tile_pool(name="Ep", bufs=1) as Ep:
                # E_sb[p, jo, i] = exp(scores[i_global, jo*128 + p] / 64)
                E_sb = Ep.tile([P, KO, S], bf16)
                qTp = tc.alloc_tile_pool(name="qTp", bufs=1)
                qT_sb = qTp.tile([P, KO, S], bf16)        # qT[d, i], released after QK
                ones_sb = persist.tile([P, 1], f32)
                b3_sb = persist.tile([P, 3 * KO], f32)
                recip_sb = persist.tile([P, NSUB], f32)   # 1/softmax-denominator
                acc_sb = persist.tile([P, S], f32)        # per-partition partial sums of E
                nc.vector.memset(ones_sb[:], 1.0)

                # Dedicated 1-buf pool for the first K block so its load can
                # prefetch while phase 1 still runs (pools reserve their SBUF
                # upfront, so the main block pool only exists in phases 2+3).
                kb0p = tc.alloc_tile_pool(name="kb0p", bufs=1)
                kb_first = [None]
                vb_first = [None]

                def _block_dma(tile, gathered, blk, n_dmas, eng=None):
                    src = gathered[blk * T:(blk + 1) * T, :].rearrange(
                        "(ko p) f -> p ko f", p=P)
                    step = KO // n_dmas
                    for i in range(n_dmas):
                        (eng or nc.sync).dma_start(
                            tile[:, i * step:(i + 1) * step, :],
                            src[:, i * step:(i + 1) * step, :],
                        )

                # Single matmul PSUM pool spanning all three phases:
                # per-phase pools would serialize each phase's first matmul
                # behind the previous phase's last PSUM drains.
                mmpsum = tc.alloc_tile_pool(name="mmpsum", bufs=6, space="PSUM")

                # ---------- Phase 1: projections kT, vT, qT ----------
                with tc.tile_pool(name="xTp", bufs=1) as xTp, \
                     tc.tile_pool(name="wp", bufs=6) as wp, \
                     tc.tile_pool(name="kvstage", bufs=6) as kvstage:
                    xT_sb = xTp.tile([P, KO, S], bf16)
                    xr = xT[:].rearrange("(ko p) f -> p ko f", p=P)
                    # Critical path: chunk 0 alone (first matmul group input),
                    # then the bulk split across the three DMA queues so the
                    # first dt-groups are never starved.
                    # Critical path on the sync HWDGE queue, in the order
                    # the first matmul group consumes it; the later half of
                    # xT rides the otherwise-idle scalar queue.
                    nc.sync.dma_start(xT_sb[:, 0, :], xr[:, 0, :])
                    nc.sync.dma_start(xT_sb[:, 1, :], xr[:, 1, :])
                    for lo in range(2, 16, 2):
                        nc.sync.dma_start(
                            xT_sb[:, lo:lo + 2, :], xr[:, lo:lo + 2, :])
                    for lo in range(16, KO, 2):
                        nc.scalar.dma_start(
                            xT_sb[:, lo:lo + 2, :], xr[:, lo:lo + 2, :])
                    nc.scalar.dma_start(b3_sb[:], b3[:])

                    # k first, then v (so their all-gathers overlap the rest
                    # of the projection compute), then q (stays in SBUF).
                    for wi, (W, bounce, boff) in enumerate((
                        (Wk, kT_bounce, KO),
                        (Wv, vT_bounce, 2 * KO),
                        (Wq, None, 0),
                    )):
                        for dt in range(KO):
                            w_sb = wp.tile([P, T], bf16, tag="w")
                            if wi == 0 and dt == 0:
                                # First two W chunks on gpsimd (empty queue,
                                # needed within ~1us of the first matmul);
                                # bulk on sync behind the critical xT pieces.
                                nc.gpsimd.dma_start(w_sb[:, 0:128], W[dt][:, 0:128])
                                nc.gpsimd.dma_start(w_sb[:, 128:256], W[dt][:, 128:256])
                                nc.sync.dma_start(w_sb[:, 256:1024], W[dt][:, 256:1024])
                                nc.sync.dma_start(w_sb[:, 1024:4096], W[dt][:, 1024:4096])
                            else:
                                nc.sync.dma_start(w_sb[:], W[dt])
                            ps = mmpsum.tile([P, S], f32, tag="mm")
                            _accum_matmuls(
                                nc, ps,
                                lambda ko, w_sb=w_sb: w_sb[:, ko * P:(ko + 1) * P],
                                lambda ko, lo, hi: xT_sb[:, ko, lo:hi],
                            )
                            bias = b3_sb[:, boff + dt:boff + dt + 1]
                            if bounce is None:
                                nc.scalar.activation(qT_sb[:, dt, :], ps[:], Ident, bias=bias)
                            else:
                                st = kvstage.tile([P, S], bf16, tag="st")
                                nc.scalar.activation(st[:], ps[:], Ident, bias=bias)
                                nc.sync.dma_start(bounce[dt * P:(dt + 1) * P, :], st[:])
                        if wi == 0:
                            nc.gpsimd.collective_compute(
                                "AllGather", mybir.AluOpType.bypass,
                                replica_groups=rg, ins=[kT_bounce[:]], outs=[kTg[:]],
                            )
                            # Prefetch the first K block on the GPSIMD queue:
                            # it sits right behind the k collective there, so
                            # its wait-on-gather parks harmlessly (on sync it
                            # would head-of-line block the phase-1 W loads —
                            # the Tile scheduler hoists it early regardless of
                            # emission position).
                            kb_first[0] = kb0p.tile([P, KO, S], bf16, name="kb0")
                            _block_dma(kb_first[0], kTg, 0, 4, eng=nc.gpsimd)
                        elif wi == 1:
                            nc.gpsimd.collective_compute(
                                "AllGather", mybir.AluOpType.bypass,
                                replica_groups=rg, ins=[vT_bounce[:]], outs=[vTg[:]],
                            )

                # ---------- Phase 2: scoresT -> E = exp(scoresT/64) ----------
                bpool = tc.alloc_tile_pool(name="blocks", bufs=3)
                with tc.tile_pool(name="spsum", bufs=2, space="PSUM") as spsum:
                    for jb in range(NB):
                        if jb == 0:
                            kb = kb_first[0]
                        else:
                            kb = bpool.tile([P, KO, S], bf16, tag="blk")
                            _block_dma(kb, kTg, jb, 4)
                        for js in range(NSUB):
                            ps = mmpsum.tile([P, S], f32, tag="mm")
                            _accum_matmuls(
                                nc, ps,
                                lambda ko, kb=kb, js=js: kb[:, ko, js * P:(js + 1) * P],
                                lambda ko, lo, hi: qT_sb[:, ko, lo:hi],
                            )
                            nc.scalar.activation(
                                E_sb[:, jb * NSUB + js, :], ps[:], Exp, scale=SCALE)
                        if jb == 0:
                            # Prefetch the first V block on the GPSIMD queue,
                            # where it sits right behind the v collective (see
                            # the K-block prefetch note above).
                            vb_first[0] = bpool.tile([P, KO, S], bf16, tag="blk", name="vb0")
                            _block_dma(vb_first[0], vTg, 0, 4, eng=nc.gpsimd)
                        Evb = E_sb[:, jb * NSUB:(jb + 1) * NSUB, :].rearrange(
                            "p ko i -> p i ko")
                        if jb == 0:
                            nc.vector.reduce_sum(
                                acc_sb[:], Evb, axis=mybir.AxisListType.X)
                        else:
                            pt = persist.tile([P, S], f32, tag="pt", bufs=2)
                            nc.vector.reduce_sum(
                                pt[:], Evb, axis=mybir.AxisListType.X)
                            nc.vector.tensor_add(acc_sb[:], acc_sb[:], pt[:])

                    for ii in range(NSUB):
                        sp = spsum.tile([P, 1], f32, tag="sum")
                        nc.tensor.matmul(
                            sp[:], acc_sb[:, ii * P:(ii + 1) * P], ones_sb[:],
                            start=True, stop=True)
                        nc.vector.reciprocal(recip_sb[:, ii:ii + 1], sp[:])

                # ---------- Phase 3: out = (E.T @ vT) / sums ----------
                with tc.tile_pool(name="ostage", bufs=4) as ostage:
                    for vb in range(NB):
                        if vb == 0:
                            vbt = vb_first[0]
                        else:
                            vbt = bpool.tile([P, KO, S], bf16, tag="blk")
                            _block_dma(vbt, vTg, vb, 4)
                        for ii in range(NSUB):
                            ps = mmpsum.tile([P, S], f32, tag="mm")
                            _accum_matmuls(
                                nc, ps,
                                lambda ko, ii=ii: E_sb[:, ko, ii * P:(ii + 1) * P],
                                lambda ko, lo, hi, vbt=vbt: vbt[:, ko, lo:hi],
                            )
                            ot = ostage.tile([P, S], f32, tag="ot")
                            nc.vector.tensor_scalar_mul(
                                ot[:], ps[:], recip_sb[:, ii:ii + 1])
                            if vb == NB - 1:
                                h = S // 2
                                nc.sync.dma_start(
                                    out[ii * P:(ii + 1) * P, vb * S:vb * S + h],
                                    ot[:, :h])
                                nc.sync.dma_start(
                                    out[ii * P:(ii + 1) * P, vb * S + h:(vb + 1) * S],
                                    ot[:, h:])
                            else:
                                nc.sync.dma_start(
                                    out[ii * P:(ii + 1) * P, vb * S:(vb + 1) * S], ot[:])
                bpool.release()
                kb0p.release()
                qTp.release()
                mmpsum.release()
    nc.compile()
    return nc


def _tile_weight(W):
    # W_t[dt, p, ko*128 + f] = W[ko*128 + p, dt*128 + f]
    W4 = np.asarray(W, dtype=np.float32).reshape(KO, P, KO, P)
    return np.ascontiguousarray(W4.transpose(2, 1, 0, 3).reshape(KO, P, T)).astype(_BF16)


def _prepare_in_maps(inputs):
    x = np.asarray(inputs["x"], dtype=np.float32)
    Wqt = _tile_weight(inputs["Wq"])
    Wkt = _tile_weight(inputs["Wk"])
    Wvt = _tile_weight(inputs["Wv"])
    b3 = np.ascontiguousarray(
        np.concatenate(
            [np.asarray(inputs[k], np.float32).reshape(KO, P).T for k in ("bq", "bk", "bv")],
            axis=1,
        )
    )
    in_maps = []
    for c in range(NCORES):
        xT_c = np.ascontiguousarray(x[c * S:(c + 1) * S, :].T).astype(_BF16)
        in_maps.append({"xT": xT_c, "Wq": Wqt, "Wk": Wkt, "Wv": Wvt, "b3": b3})
    return in_maps


def _run(inputs, trace=False, **spmd_kwargs):
    from concourse.bass_utils import run_bass_kernel_spmd

    nc = _build_program()
    in_maps = _prepare_in_maps(inputs)
    res = run_bass_kernel_spmd(
        nc, in_maps, list(range(NCORES)), trace=trace, **spmd_kwargs)
    out = np.concatenate(
        [np.asarray(res.results[c]["out"], dtype=np.float32) for c in range(NCORES)],
        axis=0,
    )
    return out, res


def kernel(**inputs):
    out, _ = _run(inputs, trace=False)
    return out
